# revision 10
# baseline (speedup 1.0000x reference)
"""Causal self-attention (B=4, T=2048, E=1024, H=16, rope) on 8 trn2 NeuronCores.

Sharding: core c = 2*b + g handles batch b = c//2, head-group g = c%2
(8 of the 16 heads).  Per core:
  - x @ Wq/Wk -> feature-major q,k with on-chip rope (fp16, FWL matmuls),
    x @ Wv -> v stored fp8 in DoubleRow kt-pair layout (ones-augmented so
    softmax denominators come out of the PV matmul for free),
  - causal attention: S^T fp16 (two heads row-tiled), exp on ACT writes
    fp8 P tiles, P.V via fp8 DoubleRow matmuls over kt-pairs; the first
    256 keys of the first q-chunk run fp16 for early-row accuracy,
  - output AllGather within the (g=0,g=1) pair (two collectives per
    512-query group), then the full row-complete output projection.
The qkv projection of chunk g+1 and the projection of group g-1 are
interleaved into attention group g so the PE stays busy while ACT runs
the exp pipeline.  Host assembles out[b] from the pair's outputs.
"""
import sys

for _p in ("/opt/trn_rl_repo", "/root/.axon_site/_ro/trn_rl_repo"):
    if _p not in sys.path:
        sys.path.append(_p)

import numpy as np
import ml_dtypes
from contextlib import ExitStack

import concourse.bass as bass
import concourse.tile as tile
from concourse import bacc, mybir
from concourse.bass_utils import run_bass_kernel_spmd

B, T, E = 4, 2048, 1024
H_TOT, D = 16, 64
HL = 8            # heads per core
F = HL * D        # 512 local q/k/v features
KB = E // 128     # 8 contraction blocks
TC = T // 512     # 4 time chunks
TT = T // 128     # 16 time tiles
NP = TT // 2      # 8 kt-pairs
ROPE_THETA = 10000.0

f32 = mybir.dt.float32
f16 = mybir.dt.float16
f8 = mybir.dt.float8e4

F8NP = ml_dtypes.float8_e4m3


def build_nc():
    nc = bacc.Bacc(None, target_bir_lowering=False, debug=False)

    x16 = nc.declare_dram_parameter("x16", [128, TC, KB, 512], f16, isOutput=False)
    wq = nc.declare_dram_parameter("wq", [128, KB, F], f16, isOutput=False)
    wk = nc.declare_dram_parameter("wk", [128, KB, F], f16, isOutput=False)
    wv = nc.declare_dram_parameter("wv", [128, KB, F], f16, isOutput=False)
    wproj = nc.declare_dram_parameter("wproj", [128, KB, E // 2], f16, isOutput=False)
    bq = nc.declare_dram_parameter("bq", [128, 4], f32, isOutput=False)
    bk = nc.declare_dram_parameter("bk", [128, 4], f32, isOutput=False)
    bvb = nc.declare_dram_parameter("bvb", [128, F], f16, isOutput=False)
    bpb = nc.declare_dram_parameter("bpb", [128, E // 2], f16, isOutput=False)
    ctab_d = nc.declare_dram_parameter("ctab", [128, T], f16, isOutput=False)
    stabp_d = nc.declare_dram_parameter("stabp", [128, T], f16, isOutput=False)
    perm_d = nc.declare_dram_parameter("perm", [128, 128], f16, isOutput=False)
    tri_d = nc.declare_dram_parameter("tri", [128, 128], f16, isOutput=False)
    out_ext = nc.declare_dram_parameter("out", [T, E // 2], f32, isOutput=True)

    GROUPS = [(0, 512), (512, 512), (1024, 512), (1536, 512)]
    # two collectives per group: half h covers head-pairs 2h, 2h+1
    ag_in = [[nc.dram_tensor(f"ag_in{i}_{h}", [128, 2, 512], f16) for h in range(2)]
             for i in range(4)]
    ag_out = [[nc.dram_tensor(f"ag_out{i}_{h}", [2, 128, 2, 512], f16) for h in range(2)]
              for i in range(4)]

    with ExitStack() as ctx:
        tc = ctx.enter_context(tile.TileContext(nc))
        sres = ctx.enter_context(tc.tile_pool(name="res", bufs=1))
        swts = ctx.enter_context(tc.tile_pool(name="wts", bufs=4))
        stab = ctx.enter_context(tc.tile_pool(name="tab", bufs=2))
        sx = ctx.enter_context(tc.tile_pool(name="x", bufs=2))
        stmp = ctx.enter_context(tc.tile_pool(name="tmp", bufs=3))
        sp = ctx.enter_context(tc.tile_pool(name="p", bufs=3))
        sof = ctx.enter_context(tc.tile_pool(name="of", bufs=2))
        sout = ctx.enter_context(tc.tile_pool(name="out", bufs=2))
        ssm = ctx.enter_context(tc.tile_pool(name="sm", bufs=2))
        pps = ctx.enter_context(tc.tile_pool(name="ps", bufs=3, space="PSUM"))
        pac = ctx.enter_context(tc.tile_pool(name="ac", bufs=2, space="PSUM"))

        # ---- resident tiles
        qT_t = sres.tile([128, 4, T], f16, tag="qT")       # rope'd q, feature-major
        kT_t = sres.tile([128, 4, T], f16, tag="kT")
        ot_t = sres.tile([128, 4, T], f16, tag="ot")       # attention out, feature-major
        # fp8 v in DoubleRow pair layout: [keys, pair, sub(kt&1), head, 80pad]
        v8_t = sres.tile([128, NP, 2, HL, 80], f8, tag="v8")
        # fp16 v for the first two key tiles (early-row accuracy)
        v16_t = sres.tile([128, 2, HL, 65], f16, tag="v16")
        nc.vector.memset(v8_t[:, :, :, :, 64:65], 1.0)
        nc.vector.memset(v16_t[:, :, :, 64:65], 1.0)

        perm_t = sres.tile([128, 128], f16, tag="perm")
        tri_t = sres.tile([128, 128], f16, tag="tri")
        bq_t = sres.tile([128, 4], f32, tag="bq")
        bk_t = sres.tile([128, 4], f32, tag="bk")
        bvb_t = sres.tile([128, F], f16, tag="bvb")
        bpb_t = sres.tile([128, E // 2], f16, tag="bpb")
        ctab_t = stab.tile([128, T], f16, tag="tab")
        stabp_t = stab.tile([128, T], f16, tag="tab")

        wq_t = swts.tile([128, KB, F], f16, tag="w")
        wk_t = swts.tile([128, KB, F], f16, tag="w")
        wv_t = swts.tile([128, KB, F], f16, tag="w")
        wp_t = swts.tile([128, KB, E // 2], f16, tag="w")

        # startup: x chunk-0 + q/k weights split in halves on the sync
        # queue (first matmul chain can start after the first pieces);
        # tables + everything else on the gpsimd queue.
        x0_t = sx.tile([128, KB, 512], f16, tag="x")
        nc.sync.dma_start(out=x0_t[:, 0:4, :], in_=x16[:, 0, 0:4, :])
        nc.sync.dma_start(out=wq_t[:, 0:4, :], in_=wq[:, 0:4, :])
        nc.sync.dma_start(out=x0_t[:, 4:8, :], in_=x16[:, 0, 4:8, :])
        nc.sync.dma_start(out=wq_t[:, 4:8, :], in_=wq[:, 4:8, :])
        nc.sync.dma_start(out=wk_t[:, 0:4, :], in_=wk[:, 0:4, :])
        nc.sync.dma_start(out=wk_t[:, 4:8, :], in_=wk[:, 4:8, :])

        nc.gpsimd.dma_start(out=ctab_t, in_=ctab_d[:, :])
        nc.gpsimd.dma_start(out=stabp_t, in_=stabp_d[:, :])
        nc.gpsimd.dma_start(out=perm_t, in_=perm_d[:, :])
        nc.gpsimd.dma_start(out=wv_t, in_=wv[:, :, :])
        nc.gpsimd.dma_start(out=tri_t, in_=tri_d[:, :])
        nc.gpsimd.dma_start(out=bq_t, in_=bq[:, :])
        nc.gpsimd.dma_start(out=bk_t, in_=bk[:, :])
        nc.gpsimd.dma_start(out=bvb_t, in_=bvb[:, :])
        nc.gpsimd.dma_start(out=bpb_t, in_=bpb[:, :])
        nc.gpsimd.dma_start(out=wp_t, in_=wproj[:, :, :])

        def load_x(tcx):
            x_t = sx.tile([128, KB, 512], f16, tag="x")
            nc.sync.dma_start(out=x_t, in_=x16[:, tcx, :, :])
            return x_t

        def qkv_chunk(tcx, x_t):
            """Generator: 8 q/k f-steps + 4 v-steps, yields between steps."""
            cs = slice(tcx * 512, (tcx + 1) * 512)
            pend = None  # (ps_p, dst, f, t1)

            def flush_perm():
                nonlocal pend
                if pend is None:
                    return
                ps_p, dst, f, t1 = pend
                nc.vector.tensor_add(dst[:, f, cs], t1[:, :], ps_p)
                pend = None

            for w_t, b_t, dst in ((wq_t, bq_t, qT_t), (wk_t, bk_t, kT_t)):
                for f in range(4):
                    ps2 = pps.tile([128, 1024], f32, tag="mm")
                    ps_q = ps2[:, 0:512]
                    ps_p = ps2[:, 512:1024]
                    for kb in range(KB):
                        nc.tensor.matmul(
                            ps_q,
                            w_t[:, kb, f * 128:(f + 1) * 128],
                            x_t[:, kb, :],
                            start=(kb == 0), stop=(kb == KB - 1),
                        )
                    # bias add on gpsimd, rope muls on DVE
                    q16 = stmp.tile([128, 512], f16, tag="q16")
                    nc.vector.tensor_scalar_add(q16[:, :], ps_q, b_t[:, f:f + 1])
                    qs = stmp.tile([128, 512], f16, tag="qs")
                    nc.vector.tensor_mul(qs[:, :], q16[:, :], stabp_t[:, cs])
                    t1 = stmp.tile([128, 512], f16, tag="t1")
                    nc.vector.tensor_mul(t1[:, :], q16[:, :], ctab_t[:, cs])
                    nc.tensor.matmul(ps_p, perm_t[:, :], qs[:, :],
                                     start=True, stop=True)
                    flush_perm()
                    pend = (ps_p, dst, f, t1)
                    yield
            flush_perm()

            for tl in range(4):
                tt = tcx * 4 + tl
                ps2 = pps.tile([128, 1024], f32, tag="mm")
                ps_v = ps2[:, 0:512]
                for kb in range(KB):
                    nc.tensor.matmul(
                        ps_v,
                        x_t[:, kb, tl * 128:(tl + 1) * 128],
                        wv_t[:, kb, :],
                        start=(kb == 0), stop=(kb == KB - 1),
                    )
                ps_v3 = ps_v.rearrange("p (h d) -> p h d", h=HL)
                bv3 = bvb_t.rearrange("p (h d) -> p h d", h=HL)
                nc.vector.tensor_add(
                    v8_t[:, tt // 2, tt % 2, :, 0:D], ps_v3, bv3)
                if tt < 2:
                    nc.vector.tensor_add(
                        v16_t[:, tt, :, 0:D], ps_v3, bv3)
                yield

        def proj_chunk(gi):
            """Generator: 4 tl units, each split in two 4-kb half-chains."""
            q0, w = GROUPS[gi]
            of_t = sof.tile([128, 2, 2, 2, 512], f16, tag="of")
            # of[p, half, s, a', t] <- ag_out[s, half, p, a', t]
            for half in range(2):
                nc.sync.dma_start(
                    out=of_t[:, half],
                    in_=ag_out[gi][half].rearrange("s p a t -> p s a t"),
                )
            for tl in range(4):
                tt = q0 // 128 + tl
                o_st = sout.tile([128, E // 2], f32, tag="o")
                ps2 = pps.tile([128, 1024], f32, tag="mm")
                ps_pj = ps2[:, 0:512]
                k = 0
                for half in range(2):
                    for s in range(2):
                        for a in range(2):
                            # contraction block (s, hp=2*half+a)
                            nc.tensor.matmul(
                                ps_pj,
                                of_t[:, half, s, a, tl * 128:(tl + 1) * 128],
                                wp_t[:, s * 4 + 2 * half + a, :],
                                start=(k == 0), stop=(k == KB - 1),
                            )
                            k += 1
                nc.vector.tensor_add(o_st[:, :], ps_pj, bpb_t[:, :])
                nc.scalar.dma_start(out=out_ext[tt * 128:(tt + 1) * 128, :],
                                    in_=o_st)
                yield

        def attn_group(gi):
            """Generator: yields after each (hp, kt-pair) step."""
            q0, w = GROUPS[gi]
            qs_sl = slice(q0, q0 + w)
            kt0 = q0 // 128
            nkt = kt0 + 4
            npair = nkt // 2

            def w0_of(kt):
                return max(kt - kt0, 0) * 128

            def emit_S(hp, kt):
                w0 = w0_of(kt)
                ps_s = pps.tile([128, 1024], f32, tag="mm")
                for half, bp in ((0, 0), (1, 64)):
                    nc.tensor.matmul(
                        ps_s[:, half * w + w0:half * w + w],
                        kT_t[bp:bp + 64, hp, kt * 128:(kt + 1) * 128],
                        qT_t[bp:bp + 64, hp, q0 + w0:q0 + w],
                        start=True, stop=True,
                    )
                return ps_s

            def emit_exp(hp, j, sub, ps_s, p8_t, p16_t):
                kt = 2 * j + sub
                w0 = w0_of(kt)
                src = ps_s.rearrange("p (h q) -> p h q", h=2)
                use16 = (gi == 0 and j == 0)
                dst_t = p16_t if use16 else p8_t
                nc.scalar.activation(
                    dst_t[:, :, sub, w0:512], src[:, :, w0:512],
                    mybir.ActivationFunctionType.Exp, scale=float(D) ** -0.5,
                )
                if kt >= kt0:  # diagonal tile: mask upper triangle (gpsimd)
                    for h in range(2):
                        ms = slice(w0, w0 + 128)
                        nc.gpsimd.tensor_mul(dst_t[:, h, sub, ms],
                                             dst_t[:, h, sub, ms], tri_t[:, :])
                if sub == 1 and kt > kt0:  # zero the odd member's gap
                    g0_, g1_ = w0_of(2 * j), w0
                    if g1_ > g0_:
                        nc.gpsimd.memset(dst_t[:, :, 1, g0_:g1_], 0.0)

            def emit_pv(hp, j, ps_os, p8_t, p16_t):
                psA, psB = ps_os
                w0 = w0_of(2 * j)
                stop = (j == npair - 1)
                if gi == 0 and j == 0:
                    for kt in (0, 1):
                        kw0 = w0_of(kt)
                        for h, ps_o in ((0, psA), (1, psB)):
                            nc.tensor.matmul(
                                ps_o[:, kw0:512],
                                v16_t[:, kt, 2 * hp + h, :],
                                p16_t[:, h, kt, kw0:512],
                                start=(kt == 0), stop=(stop and kt == 1),
                            )
                else:
                    for h, ps_o in ((0, psA), (1, psB)):
                        nc.tensor.matmul(
                            ps_o[:, w0:512],
                            v8_t[:, j, :, 2 * hp + h, 0:65],
                            p8_t[:, h, :, w0:512],
                            start=(j == 0), stop=stop,
                            perf_mode=mybir.MatmulPerfMode.DoubleRow,
                        )

            def finish_hp(hp, ps_os):
                for bp, ps_o in ((0, ps_os[0]), (64, ps_os[1])):
                    sums_sb = ssm.tile([1, 512], f32, tag="sums")
                    nc.vector.tensor_copy(sums_sb[:, :], ps_o[D:D + 1, :])
                    recip = ssm.tile([1, 512], f32, tag="rc")
                    nc.vector.reciprocal_approx_fast(out=recip[:, :],
                                                     in_=sums_sb[:, :])
                    bc = ssm.tile([64, 512], f32, tag="bc")
                    nc.gpsimd.partition_broadcast(bc[:, :], recip[:, :])
                    nc.vector.tensor_mul(ot_t[bp:bp + 64, hp, qs_sl],
                                         ps_o[0:D, :], bc[:, :])
                # stage this head-pair's gathered strip
                nc.gpsimd.dma_start(
                    out=ag_in[gi][hp // 2][:, hp % 2, :],
                    in_=ot_t[:, hp, q0:q0 + w])
                if hp % 2 == 1:
                    nc.gpsimd.collective_compute(
                        "AllGather",
                        mybir.AluOpType.bypass,
                        ins=[ag_in[gi][hp // 2][:, :, :]],
                        outs=[ag_out[gi][hp // 2][:, :, :, :]],
                        replica_groups=[[0, 1], [2, 3], [4, 5], [6, 7]],
                    )

            for hp in range(HL // 2):
                ps_os = (pac.tile([65, 512], f32, tag="acc", name="oA"),
                         pac.tile([65, 512], f32, tag="acc", name="oB"))
                for j in range(npair):
                    p8_t = sp.tile([128, 2, 2, 512], f8, tag="p8", name="p8_t")
                    p16_t = None
                    if gi == 0 and j == 0:
                        p16_t = sp.tile([128, 2, 2, 512], f16, tag="p16",
                                        name="p16_t")
                    ps_e = emit_S(hp, 2 * j)
                    ps_o2 = emit_S(hp, 2 * j + 1)
                    emit_exp(hp, j, 0, ps_e, p8_t, p16_t)
                    emit_exp(hp, j, 1, ps_o2, p8_t, p16_t)
                    # fillers land between exp and PV so the PE has work
                    # while ACT runs the exp pipeline
                    yield
                    emit_pv(hp, j, ps_os, p8_t, p16_t)
                finish_hp(hp, ps_os)

        # ---------- interleaved schedule ----------
        # attention group g runs with qkv chunk g+1 and projection of
        # group g-1 woven between its pair-steps.
        for _ in qkv_chunk(0, x0_t):
            pass
        x_next = load_x(1)
        for gi in range(TC):
            filler_gens = []
            nfill = 0
            if gi + 1 < TC:
                filler_gens.append(qkv_chunk(gi + 1, x_next))
                nfill += 12
            if gi >= 1:
                filler_gens.append(proj_chunk(gi - 1))
                nfill += 4
            if gi + 2 < TC:
                x_next = load_x(gi + 2)
            npairs = 4 * 2 * (gi + 1)
            emitted = 0
            step = 0
            for _ in attn_group(gi):
                step += 1
                want = min(nfill, (step * nfill) // max(npairs - 2, 1))
                while emitted < want and filler_gens:
                    try:
                        next(filler_gens[0])
                        emitted += 1
                    except StopIteration:
                        filler_gens.pop(0)
            while filler_gens:
                try:
                    next(filler_gens[0])
                    emitted += 1
                except StopIteration:
                    filler_gens.pop(0)
        for _ in proj_chunk(3):
            pass

    nc.compile()
    return nc


_NC = None


def _get_nc():
    global _NC
    if _NC is None:
        _NC = build_nc()
    return _NC


def _host_prep(x, Wqkv, bqkv, Wproj, bproj):
    """Build the 8 per-core input maps."""
    x = np.asarray(x, np.float32)
    Wqkv = np.asarray(Wqkv, np.float32)
    bqkv = np.asarray(bqkv, np.float32)
    Wproj = np.asarray(Wproj, np.float32)
    bproj = np.asarray(bproj, np.float32)

    perm_idx = np.concatenate([np.arange(0, D, 2), np.arange(1, D, 2)])

    inv_freq = 1.0 / ROPE_THETA ** (np.arange(0, D, 2, dtype=np.float32) / D)
    freqs = np.arange(T, dtype=np.float32)[:, None] * inv_freq[None, :]
    cosf = np.cos(freqs).T.astype(np.float32)  # (32, T)
    sinf = np.sin(freqs).T.astype(np.float32)
    ctab = np.tile(cosf, (4, 1)).astype(np.float16)                 # (128, T)
    stab = np.concatenate([-sinf, sinf, -sinf, sinf], 0).astype(np.float16)
    pi = np.array([(m // 64) * 64 + (m % 64 + 32) % 64 for m in range(128)])
    stabp = stab[pi]

    pmat = np.zeros((128, 128), np.float16)
    for m in range(128):
        pmat[pi[m], m] = 1.0

    tri = (np.arange(128)[:, None] <= np.arange(128)[None, :]).astype(np.float16)

    maps = []
    for c in range(8):
        b, g = c // 2, c % 2
        heads = np.arange(8 * g, 8 * g + 8)
        qcols = (heads[:, None] * D + perm_idx[None, :]).ravel()
        vcols = (heads[:, None] * D + np.arange(D)[None, :]).ravel()

        xb = x[b].astype(np.float16)                      # (T, E)
        x16 = xb.reshape(TC, 512, KB, 128).transpose(3, 0, 2, 1)

        def wmat(cols):
            wm = Wqkv[:, cols].astype(np.float16)          # (E, F)
            return wm.reshape(KB, 128, F).transpose(1, 0, 2)

        # wproj rows ordered to match proj contraction blocks kb=(s, hp):
        # feature(s, hp, p) = (8s + 2*hp + (p>=64))*64 + p%64
        s_i, a_i, p_i = np.meshgrid(np.arange(2), np.arange(4), np.arange(128),
                                    indexing="ij")
        feat = (8 * s_i + 2 * a_i + (p_i >= 64)) * 64 + p_i % 64
        wp = Wproj[:, g * 512:(g + 1) * 512].astype(np.float16)
        wp16 = wp[feat.reshape(8, 128)].transpose(1, 0, 2)

        maps.append({
            "x16": np.ascontiguousarray(x16),
            "wq": np.ascontiguousarray(wmat(qcols)),
            "wk": np.ascontiguousarray(wmat(E + qcols)),
            "wv": np.ascontiguousarray(wmat(2 * E + vcols)),
            "wproj": np.ascontiguousarray(wp16),
            "bq": np.ascontiguousarray(bqkv[qcols].reshape(4, 128).T.astype(np.float32)),
            "bk": np.ascontiguousarray(bqkv[E + qcols].reshape(4, 128).T.astype(np.float32)),
            "bvb": np.ascontiguousarray(
                np.tile(bqkv[2 * E + vcols].astype(np.float16)[None, :], (128, 1))),
            "bpb": np.ascontiguousarray(
                np.tile(bproj[g * 512:(g + 1) * 512].astype(np.float16)[None, :], (128, 1))),
            "ctab": ctab,
            "stabp": np.ascontiguousarray(stabp),
            "perm": pmat,
            "tri": tri,
        })
    return maps


def kernel(x, Wqkv, bqkv, Wproj, bproj):
    nc = _get_nc()
    in_maps = _host_prep(x, Wqkv, bqkv, Wproj, bproj)
    res = run_bass_kernel_spmd(nc, in_maps, list(range(8)))
    out = np.empty((B, T, E), np.float32)
    for b in range(B):
        out[b, :, :E // 2] = res.results[2 * b]["out"]
        out[b, :, E // 2:] = res.results[2 * b + 1]["out"]
    return out


if __name__ == "__main__":
    rng = np.random.default_rng(0)
    x = rng.standard_normal((B, T, E), dtype=np.float32)
    Wqkv = rng.standard_normal((E, 3 * E), dtype=np.float32) * 0.02
    bqkv = rng.standard_normal((3 * E,), dtype=np.float32) * 0.02
    Wproj = rng.standard_normal((E, E), dtype=np.float32) * 0.02
    bproj = rng.standard_normal((E,), dtype=np.float32) * 0.02
    o = kernel(x=x, Wqkv=Wqkv, bqkv=bqkv, Wproj=Wproj, bproj=bproj)
    print("out", o.shape, o.dtype, float(np.abs(o).max()))


# revision 11
# speedup vs baseline: 1.3465x; 1.3465x over previous
"""Causal self-attention (B=4, T=2048, E=1024, H=16, rope) on 8 trn2 NeuronCores.

Sharding: core c = 2*b + g handles batch b = c//2, head-group g = c%2
(8 of the 16 heads).  Per core:
  - x @ Wq/Wk -> feature-major q,k with on-chip rope (fp16, FWL matmuls),
    x @ Wv -> v stored fp8 in DoubleRow kt-pair layout (ones-augmented so
    softmax denominators come out of the PV matmul for free),
  - causal attention: S^T fp16 (two heads row-tiled), exp on ACT writes
    fp8 P tiles, P.V via fp8 DoubleRow matmuls over kt-pairs; the first
    256 keys of the first q-chunk run fp16 for early-row accuracy,
  - output AllGather within the (g=0,g=1) pair (two collectives per
    512-query group), then the full row-complete output projection.
The qkv projection of chunk g+1 and the projection of group g-1 are
interleaved into attention group g so the PE stays busy while ACT runs
the exp pipeline.  Host assembles out[b] from the pair's outputs.
"""
import sys

for _p in ("/opt/trn_rl_repo", "/root/.axon_site/_ro/trn_rl_repo"):
    if _p not in sys.path:
        sys.path.append(_p)

import numpy as np
import ml_dtypes
from contextlib import ExitStack

import concourse.bass as bass
import concourse.tile as tile
from concourse import bacc, mybir
from concourse.bass_utils import run_bass_kernel_spmd

B, T, E = 4, 2048, 1024
H_TOT, D = 16, 64
HL = 8            # heads per core
F = HL * D        # 512 local q/k/v features
KB = E // 128     # 8 contraction blocks
TC = T // 512     # 4 time chunks
TT = T // 128     # 16 time tiles
NP = TT // 2      # 8 kt-pairs
ROPE_THETA = 10000.0

f32 = mybir.dt.float32
f16 = mybir.dt.float16
f8 = mybir.dt.float8e4

F8NP = ml_dtypes.float8_e4m3


def build_nc():
    nc = bacc.Bacc(None, target_bir_lowering=False, debug=False)

    x16 = nc.declare_dram_parameter("x16", [128, TC, KB, 512], f16, isOutput=False)
    wq = nc.declare_dram_parameter("wq", [128, KB, F], f16, isOutput=False)
    wk = nc.declare_dram_parameter("wk", [128, KB, F], f16, isOutput=False)
    wv = nc.declare_dram_parameter("wv", [128, KB, F], f16, isOutput=False)
    wproj = nc.declare_dram_parameter("wproj", [128, KB, E // 2], f16, isOutput=False)
    bq = nc.declare_dram_parameter("bq", [128, 4], f32, isOutput=False)
    bk = nc.declare_dram_parameter("bk", [128, 4], f32, isOutput=False)
    bvb = nc.declare_dram_parameter("bvb", [128, F], f16, isOutput=False)
    bpb = nc.declare_dram_parameter("bpb", [128, E // 2], f16, isOutput=False)
    ctab_d = nc.declare_dram_parameter("ctab", [128, T], f16, isOutput=False)
    stabp_d = nc.declare_dram_parameter("stabp", [128, T], f16, isOutput=False)
    perm_d = nc.declare_dram_parameter("perm", [128, 128], f16, isOutput=False)
    tri_d = nc.declare_dram_parameter("tri", [128, 128], f16, isOutput=False)
    out_ext = nc.declare_dram_parameter("out", [T, E // 2], f32, isOutput=True)

    GROUPS = [(0, 512), (512, 512), (1024, 512), (1536, 512)]
    # two collectives per group: half h covers head-pairs 2h, 2h+1
    ag_in = [[nc.dram_tensor(f"ag_in{i}_{h}", [128, 2, 512], f16) for h in range(2)]
             for i in range(4)]
    ag_out = [[nc.dram_tensor(f"ag_out{i}_{h}", [2, 128, 2, 512], f16) for h in range(2)]
              for i in range(4)]

    with ExitStack() as ctx:
        tc = ctx.enter_context(tile.TileContext(nc))
        sres = ctx.enter_context(tc.tile_pool(name="res", bufs=1))
        swts = ctx.enter_context(tc.tile_pool(name="wts", bufs=4))
        stab = ctx.enter_context(tc.tile_pool(name="tab", bufs=2))
        sx = ctx.enter_context(tc.tile_pool(name="x", bufs=2))
        stmp = ctx.enter_context(tc.tile_pool(name="tmp", bufs=3))
        sp = ctx.enter_context(tc.tile_pool(name="p", bufs=3))
        sof = ctx.enter_context(tc.tile_pool(name="of", bufs=2))
        sout = ctx.enter_context(tc.tile_pool(name="out", bufs=2))
        ssm = ctx.enter_context(tc.tile_pool(name="sm", bufs=2))
        pps = ctx.enter_context(tc.tile_pool(name="ps", bufs=3, space="PSUM"))
        pac = ctx.enter_context(tc.tile_pool(name="ac", bufs=2, space="PSUM"))

        # ---- resident tiles
        qT_t = sres.tile([128, 4, T], f16, tag="qT")       # rope'd q, feature-major
        kT_t = sres.tile([128, 4, T], f16, tag="kT")
        ot_t = sres.tile([128, 4, T], f16, tag="ot")       # attention out, feature-major
        # fp8 v in DoubleRow pair layout: [keys, pair, sub(kt&1), head, 80pad]
        v8_t = sres.tile([128, NP, 2, HL, 80], f8, tag="v8")
        # fp16 v for the first two key tiles (early-row accuracy)
        v16_t = sres.tile([128, 2, HL, 65], f16, tag="v16")
        nc.vector.memset(v8_t[:, :, :, :, 64:65], 1.0)
        nc.vector.memset(v16_t[:, :, :, 64:65], 1.0)

        perm_t = sres.tile([128, 128], f16, tag="perm")
        tri_t = sres.tile([128, 128], f16, tag="tri")
        bq_t = sres.tile([128, 4], f32, tag="bq")
        bk_t = sres.tile([128, 4], f32, tag="bk")
        bvb_t = sres.tile([128, F], f16, tag="bvb")
        bpb_t = sres.tile([128, E // 2], f16, tag="bpb")
        ctab_t = stab.tile([128, T], f16, tag="tab")
        stabp_t = stab.tile([128, T], f16, tag="tab")

        wq_t = swts.tile([128, KB, F], f16, tag="w")
        wk_t = swts.tile([128, KB, F], f16, tag="w")
        wv_t = swts.tile([128, KB, F], f16, tag="w")
        wp_t = swts.tile([128, KB, E // 2], f16, tag="w")

        # startup: x chunk-0 + q/k weights split in halves on the sync
        # queue (first matmul chain can start after the first pieces);
        # tables + everything else on the gpsimd queue.
        x0_t = sx.tile([128, KB, 512], f16, tag="x")
        nc.sync.dma_start(out=x0_t[:, 0:4, :], in_=x16[:, 0, 0:4, :])
        nc.sync.dma_start(out=wq_t[:, 0:4, :], in_=wq[:, 0:4, :])
        nc.sync.dma_start(out=x0_t[:, 4:8, :], in_=x16[:, 0, 4:8, :])
        nc.sync.dma_start(out=wq_t[:, 4:8, :], in_=wq[:, 4:8, :])
        nc.sync.dma_start(out=wk_t[:, 0:4, :], in_=wk[:, 0:4, :])
        nc.sync.dma_start(out=wk_t[:, 4:8, :], in_=wk[:, 4:8, :])

        nc.gpsimd.dma_start(out=ctab_t, in_=ctab_d[:, :])
        nc.gpsimd.dma_start(out=stabp_t, in_=stabp_d[:, :])
        nc.gpsimd.dma_start(out=perm_t, in_=perm_d[:, :])
        nc.gpsimd.dma_start(out=wv_t, in_=wv[:, :, :])
        nc.gpsimd.dma_start(out=tri_t, in_=tri_d[:, :])
        nc.gpsimd.dma_start(out=bq_t, in_=bq[:, :])
        nc.gpsimd.dma_start(out=bk_t, in_=bk[:, :])
        nc.gpsimd.dma_start(out=bvb_t, in_=bvb[:, :])
        nc.gpsimd.dma_start(out=bpb_t, in_=bpb[:, :])
        nc.gpsimd.dma_start(out=wp_t, in_=wproj[:, :, :])

        def load_x(tcx):
            x_t = sx.tile([128, KB, 512], f16, tag="x")
            nc.sync.dma_start(out=x_t, in_=x16[:, tcx, :, :])
            return x_t

        def qkv_chunk(tcx, x_t):
            """Generator: 8 q/k f-steps + 4 v-steps, yields between steps."""
            cs = slice(tcx * 512, (tcx + 1) * 512)
            pend = None  # (ps_p, dst, f, t1)

            def flush_perm():
                nonlocal pend
                if pend is None:
                    return
                ps_p, dst, f, t1 = pend
                nc.vector.tensor_add(dst[:, f, cs], t1[:, :], ps_p)
                pend = None

            for w_t, b_t, dst in ((wq_t, bq_t, qT_t), (wk_t, bk_t, kT_t)):
                for f in range(4):
                    ps2 = pps.tile([128, 1024], f32, tag="mm")
                    ps_q = ps2[:, 0:512]
                    ps_p = ps2[:, 512:1024]
                    for kb in range(KB):
                        nc.tensor.matmul(
                            ps_q,
                            w_t[:, kb, f * 128:(f + 1) * 128],
                            x_t[:, kb, :],
                            start=(kb == 0), stop=(kb == KB - 1),
                        )
                    # bias add on gpsimd, rope muls on DVE
                    q16 = stmp.tile([128, 512], f16, tag="q16")
                    nc.vector.tensor_scalar_add(q16[:, :], ps_q, b_t[:, f:f + 1])
                    qs = stmp.tile([128, 512], f16, tag="qs")
                    nc.vector.tensor_mul(qs[:, :], q16[:, :], stabp_t[:, cs])
                    t1 = stmp.tile([128, 512], f16, tag="t1")
                    nc.vector.tensor_mul(t1[:, :], q16[:, :], ctab_t[:, cs])
                    nc.tensor.matmul(ps_p, perm_t[:, :], qs[:, :],
                                     start=True, stop=True)
                    flush_perm()
                    pend = (ps_p, dst, f, t1)
                    yield
            flush_perm()

            for tl in range(4):
                tt = tcx * 4 + tl
                ps2 = pps.tile([128, 1024], f32, tag="mm")
                ps_v = ps2[:, 0:512]
                for kb in range(KB):
                    nc.tensor.matmul(
                        ps_v,
                        x_t[:, kb, tl * 128:(tl + 1) * 128],
                        wv_t[:, kb, :],
                        start=(kb == 0), stop=(kb == KB - 1),
                    )
                ps_v3 = ps_v.rearrange("p (h d) -> p h d", h=HL)
                bv3 = bvb_t.rearrange("p (h d) -> p h d", h=HL)
                nc.vector.tensor_add(
                    v8_t[:, tt // 2, tt % 2, :, 0:D], ps_v3, bv3)
                if tt < 2:
                    nc.vector.tensor_add(
                        v16_t[:, tt, :, 0:D], ps_v3, bv3)
                yield

        def proj_chunk(gi):
            """Generator: 4 tl units, each split in two 4-kb half-chains."""
            q0, w = GROUPS[gi]
            of_t = sof.tile([128, 2, 2, 2, 512], f16, tag="of")
            # of[p, half, s, a', t] <- ag_out[s, half, p, a', t]
            for half in range(2):
                nc.sync.dma_start(
                    out=of_t[:, half],
                    in_=ag_out[gi][half].rearrange("s p a t -> p s a t"),
                )
            for tl in range(4):
                tt = q0 // 128 + tl
                o_st = sout.tile([128, E // 2], f32, tag="o")
                ps2 = pps.tile([128, 1024], f32, tag="mm")
                ps_pj = ps2[:, 0:512]
                k = 0
                for half in range(2):
                    for s in range(2):
                        for a in range(2):
                            # contraction block (s, hp=2*half+a)
                            nc.tensor.matmul(
                                ps_pj,
                                of_t[:, half, s, a, tl * 128:(tl + 1) * 128],
                                wp_t[:, s * 4 + 2 * half + a, :],
                                start=(k == 0), stop=(k == KB - 1),
                            )
                            k += 1
                nc.vector.tensor_add(o_st[:, :], ps_pj, bpb_t[:, :])
                nc.scalar.dma_start(out=out_ext[tt * 128:(tt + 1) * 128, :],
                                    in_=o_st)
                yield

        def attn_group(gi):
            """Generator: yields after each (hp, kt-pair) step."""
            q0, w = GROUPS[gi]
            qs_sl = slice(q0, q0 + w)
            kt0 = q0 // 128
            nkt = kt0 + 4
            npair = nkt // 2

            def w0_of(kt):
                return max(kt - kt0, 0) * 128

            def emit_S(hp, kt):
                w0 = w0_of(kt)
                ps_s = pps.tile([128, 1024], f32, tag="mm")
                for half, bp in ((0, 0), (1, 64)):
                    nc.tensor.matmul(
                        ps_s[:, half * w + w0:half * w + w],
                        kT_t[bp:bp + 64, hp, kt * 128:(kt + 1) * 128],
                        qT_t[bp:bp + 64, hp, q0 + w0:q0 + w],
                        start=True, stop=True,
                    )
                return ps_s

            def emit_exp(hp, j, sub, ps_s, p8_t, p16_t):
                kt = 2 * j + sub
                w0 = w0_of(kt)
                src = ps_s.rearrange("p (h q) -> p h q", h=2)
                use16 = (gi == 0 and j == 0)
                dst_t = p16_t if use16 else p8_t
                nc.scalar.activation(
                    dst_t[:, :, sub, w0:512], src[:, :, w0:512],
                    mybir.ActivationFunctionType.Exp, scale=float(D) ** -0.5,
                )
                if kt >= kt0:  # diagonal tile: mask upper triangle
                    for h in range(2):
                        ms = slice(w0, w0 + 128)
                        nc.vector.tensor_mul(dst_t[:, h, sub, ms],
                                             dst_t[:, h, sub, ms], tri_t[:, :])
                if sub == 1 and kt > kt0:  # zero the odd member's gap
                    g0_, g1_ = w0_of(2 * j), w0
                    if g1_ > g0_:
                        nc.vector.memset(dst_t[:, :, 1, g0_:g1_], 0.0)

            def emit_pv(hp, j, ps_os, p8_t, p16_t):
                psA, psB = ps_os
                w0 = w0_of(2 * j)
                stop = (j == npair - 1)
                if gi == 0 and j == 0:
                    for kt in (0, 1):
                        kw0 = w0_of(kt)
                        for h, ps_o in ((0, psA), (1, psB)):
                            nc.tensor.matmul(
                                ps_o[:, kw0:512],
                                v16_t[:, kt, 2 * hp + h, :],
                                p16_t[:, h, kt, kw0:512],
                                start=(kt == 0), stop=(stop and kt == 1),
                            )
                else:
                    for h, ps_o in ((0, psA), (1, psB)):
                        nc.tensor.matmul(
                            ps_o[:, w0:512],
                            v8_t[:, j, :, 2 * hp + h, 0:65],
                            p8_t[:, h, :, w0:512],
                            start=(j == 0), stop=stop,
                            perf_mode=mybir.MatmulPerfMode.DoubleRow,
                        )

            def finish_hp(hp, ps_os):
                for bp, ps_o in ((0, ps_os[0]), (64, ps_os[1])):
                    sums_sb = ssm.tile([1, 512], f32, tag="sums")
                    nc.vector.tensor_copy(sums_sb[:, :], ps_o[D:D + 1, :])
                    recip = ssm.tile([1, 512], f32, tag="rc")
                    nc.vector.reciprocal_approx_fast(out=recip[:, :],
                                                     in_=sums_sb[:, :])
                    bc = ssm.tile([64, 512], f32, tag="bc")
                    nc.gpsimd.partition_broadcast(bc[:, :], recip[:, :])
                    nc.vector.tensor_mul(ot_t[bp:bp + 64, hp, qs_sl],
                                         ps_o[0:D, :], bc[:, :])
                # stage this head-pair's gathered strip
                nc.scalar.dma_start(
                    out=ag_in[gi][hp // 2][:, hp % 2, :],
                    in_=ot_t[:, hp, q0:q0 + w])
                if hp % 2 == 1:
                    nc.gpsimd.collective_compute(
                        "AllGather",
                        mybir.AluOpType.bypass,
                        ins=[ag_in[gi][hp // 2][:, :, :]],
                        outs=[ag_out[gi][hp // 2][:, :, :, :]],
                        replica_groups=[[0, 1], [2, 3], [4, 5], [6, 7]],
                    )

            for hp in range(HL // 2):
                ps_os = (pac.tile([65, 512], f32, tag="acc", name="oA"),
                         pac.tile([65, 512], f32, tag="acc", name="oB"))
                for j in range(npair):
                    p8_t = sp.tile([128, 2, 2, 512], f8, tag="p8", name="p8_t")
                    p16_t = None
                    if gi == 0 and j == 0:
                        p16_t = sp.tile([128, 2, 2, 512], f16, tag="p16",
                                        name="p16_t")
                    ps_e = emit_S(hp, 2 * j)
                    ps_o2 = emit_S(hp, 2 * j + 1)
                    emit_exp(hp, j, 0, ps_e, p8_t, p16_t)
                    emit_exp(hp, j, 1, ps_o2, p8_t, p16_t)
                    # fillers land between exp and PV so the PE has work
                    # while ACT runs the exp pipeline
                    yield
                    emit_pv(hp, j, ps_os, p8_t, p16_t)
                finish_hp(hp, ps_os)

        # ---------- interleaved schedule ----------
        # attention group g runs with qkv chunk g+1 and projection of
        # group g-1 woven between its pair-steps.
        for _ in qkv_chunk(0, x0_t):
            pass
        x_next = load_x(1)
        for gi in range(TC):
            filler_gens = []
            nfill = 0
            if gi + 1 < TC:
                filler_gens.append(qkv_chunk(gi + 1, x_next))
                nfill += 12
            if gi >= 1:
                filler_gens.append(proj_chunk(gi - 1))
                nfill += 4
            if gi + 2 < TC:
                x_next = load_x(gi + 2)
            npairs = 4 * 2 * (gi + 1)
            emitted = 0
            step = 0
            for _ in attn_group(gi):
                step += 1
                want = min(nfill, (step * nfill) // max(npairs - 2, 1))
                while emitted < want and filler_gens:
                    try:
                        next(filler_gens[0])
                        emitted += 1
                    except StopIteration:
                        filler_gens.pop(0)
            while filler_gens:
                try:
                    next(filler_gens[0])
                    emitted += 1
                except StopIteration:
                    filler_gens.pop(0)
        for _ in proj_chunk(3):
            pass

    nc.compile()
    return nc


_NC = None


def _get_nc():
    global _NC
    if _NC is None:
        _NC = build_nc()
    return _NC


def _host_prep(x, Wqkv, bqkv, Wproj, bproj):
    """Build the 8 per-core input maps."""
    x = np.asarray(x, np.float32)
    Wqkv = np.asarray(Wqkv, np.float32)
    bqkv = np.asarray(bqkv, np.float32)
    Wproj = np.asarray(Wproj, np.float32)
    bproj = np.asarray(bproj, np.float32)

    perm_idx = np.concatenate([np.arange(0, D, 2), np.arange(1, D, 2)])

    inv_freq = 1.0 / ROPE_THETA ** (np.arange(0, D, 2, dtype=np.float32) / D)
    freqs = np.arange(T, dtype=np.float32)[:, None] * inv_freq[None, :]
    cosf = np.cos(freqs).T.astype(np.float32)  # (32, T)
    sinf = np.sin(freqs).T.astype(np.float32)
    ctab = np.tile(cosf, (4, 1)).astype(np.float16)                 # (128, T)
    stab = np.concatenate([-sinf, sinf, -sinf, sinf], 0).astype(np.float16)
    pi = np.array([(m // 64) * 64 + (m % 64 + 32) % 64 for m in range(128)])
    stabp = stab[pi]

    pmat = np.zeros((128, 128), np.float16)
    for m in range(128):
        pmat[pi[m], m] = 1.0

    tri = (np.arange(128)[:, None] <= np.arange(128)[None, :]).astype(np.float16)

    maps = []
    for c in range(8):
        b, g = c // 2, c % 2
        heads = np.arange(8 * g, 8 * g + 8)
        qcols = (heads[:, None] * D + perm_idx[None, :]).ravel()
        vcols = (heads[:, None] * D + np.arange(D)[None, :]).ravel()

        xb = x[b].astype(np.float16)                      # (T, E)
        x16 = xb.reshape(TC, 512, KB, 128).transpose(3, 0, 2, 1)

        def wmat(cols):
            wm = Wqkv[:, cols].astype(np.float16)          # (E, F)
            return wm.reshape(KB, 128, F).transpose(1, 0, 2)

        # wproj rows ordered to match proj contraction blocks kb=(s, hp):
        # feature(s, hp, p) = (8s + 2*hp + (p>=64))*64 + p%64
        s_i, a_i, p_i = np.meshgrid(np.arange(2), np.arange(4), np.arange(128),
                                    indexing="ij")
        feat = (8 * s_i + 2 * a_i + (p_i >= 64)) * 64 + p_i % 64
        wp = Wproj[:, g * 512:(g + 1) * 512].astype(np.float16)
        wp16 = wp[feat.reshape(8, 128)].transpose(1, 0, 2)

        maps.append({
            "x16": np.ascontiguousarray(x16),
            "wq": np.ascontiguousarray(wmat(qcols)),
            "wk": np.ascontiguousarray(wmat(E + qcols)),
            "wv": np.ascontiguousarray(wmat(2 * E + vcols)),
            "wproj": np.ascontiguousarray(wp16),
            "bq": np.ascontiguousarray(bqkv[qcols].reshape(4, 128).T.astype(np.float32)),
            "bk": np.ascontiguousarray(bqkv[E + qcols].reshape(4, 128).T.astype(np.float32)),
            "bvb": np.ascontiguousarray(
                np.tile(bqkv[2 * E + vcols].astype(np.float16)[None, :], (128, 1))),
            "bpb": np.ascontiguousarray(
                np.tile(bproj[g * 512:(g + 1) * 512].astype(np.float16)[None, :], (128, 1))),
            "ctab": ctab,
            "stabp": np.ascontiguousarray(stabp),
            "perm": pmat,
            "tri": tri,
        })
    return maps


def kernel(x, Wqkv, bqkv, Wproj, bproj):
    nc = _get_nc()
    in_maps = _host_prep(x, Wqkv, bqkv, Wproj, bproj)
    res = run_bass_kernel_spmd(nc, in_maps, list(range(8)))
    out = np.empty((B, T, E), np.float32)
    for b in range(B):
        out[b, :, :E // 2] = res.results[2 * b]["out"]
        out[b, :, E // 2:] = res.results[2 * b + 1]["out"]
    return out


if __name__ == "__main__":
    rng = np.random.default_rng(0)
    x = rng.standard_normal((B, T, E), dtype=np.float32)
    Wqkv = rng.standard_normal((E, 3 * E), dtype=np.float32) * 0.02
    bqkv = rng.standard_normal((3 * E,), dtype=np.float32) * 0.02
    Wproj = rng.standard_normal((E, E), dtype=np.float32) * 0.02
    bproj = rng.standard_normal((E,), dtype=np.float32) * 0.02
    o = kernel(x=x, Wqkv=Wqkv, bqkv=bqkv, Wproj=Wproj, bproj=bproj)
    print("out", o.shape, o.dtype, float(np.abs(o).max()))


# revision 13
# speedup vs baseline: 1.4113x; 1.0482x over previous
"""Causal self-attention (B=4, T=2048, E=1024, H=16, rope) on 8 trn2 NeuronCores.

Sharding: core c = 2*b + g handles batch b = c//2, head-group g = c%2
(8 of the 16 heads).  Per core:
  - x @ Wq/Wk -> feature-major q,k with on-chip rope (fp16, FWL matmuls),
    x @ Wv -> v stored fp8 in DoubleRow kt-pair layout (ones-augmented so
    softmax denominators come out of the PV matmul for free),
  - causal attention: S^T fp16 (two heads row-tiled), exp on ACT writes
    fp8 P tiles, P.V via fp8 DoubleRow matmuls over kt-pairs; the first
    256 keys of the first q-chunk run fp16 for early-row accuracy,
  - output AllGather within the (g=0,g=1) pair (two collectives per
    512-query group), then the full row-complete output projection.
The qkv projection of chunk g+1 and the projection of group g-1 are
interleaved into attention group g so the PE stays busy while ACT runs
the exp pipeline.  Host assembles out[b] from the pair's outputs.
"""
import sys

for _p in ("/opt/trn_rl_repo", "/root/.axon_site/_ro/trn_rl_repo"):
    if _p not in sys.path:
        sys.path.append(_p)

import numpy as np
import ml_dtypes
from contextlib import ExitStack

import concourse.bass as bass
import concourse.tile as tile
from concourse import bacc, mybir
from concourse.bass_utils import run_bass_kernel_spmd

B, T, E = 4, 2048, 1024
H_TOT, D = 16, 64
HL = 8            # heads per core
F = HL * D        # 512 local q/k/v features
KB = E // 128     # 8 contraction blocks
TC = T // 512     # 4 time chunks
TT = T // 128     # 16 time tiles
NP = TT // 2      # 8 kt-pairs
ROPE_THETA = 10000.0

f32 = mybir.dt.float32
f16 = mybir.dt.float16
f8 = mybir.dt.float8e4

F8NP = ml_dtypes.float8_e4m3


def build_nc():
    nc = bacc.Bacc(None, target_bir_lowering=False, debug=False)

    x16 = nc.declare_dram_parameter("x16", [128, TC, KB, 512], f16, isOutput=False)
    wq = nc.declare_dram_parameter("wq", [128, 4, KB, 128], f16, isOutput=False)
    wk = nc.declare_dram_parameter("wk", [128, 4, KB, 128], f16, isOutput=False)
    wv = nc.declare_dram_parameter("wv", [128, KB, F], f16, isOutput=False)
    wproj = nc.declare_dram_parameter("wproj", [128, KB, E // 2], f16, isOutput=False)
    bq = nc.declare_dram_parameter("bq", [128, 4], f32, isOutput=False)
    bk = nc.declare_dram_parameter("bk", [128, 4], f32, isOutput=False)
    bvb = nc.declare_dram_parameter("bvb", [128, F], f16, isOutput=False)
    bpb = nc.declare_dram_parameter("bpb", [128, E // 2], f16, isOutput=False)
    ctab_d = nc.declare_dram_parameter("ctab", [128, T], f16, isOutput=False)
    stabp_d = nc.declare_dram_parameter("stabp", [128, T], f16, isOutput=False)
    perm_d = nc.declare_dram_parameter("perm", [128, 128], f16, isOutput=False)
    tri_d = nc.declare_dram_parameter("tri", [128, 128], f16, isOutput=False)
    out_ext = nc.declare_dram_parameter("out", [T, E // 2], f32, isOutput=True)

    GROUPS = [(0, 512), (512, 512), (1024, 512), (1536, 512)]
    ag_in = [nc.dram_tensor(f"ag_in{i}", [128, 4, 512], f16) for i in range(4)]
    ag_out = [nc.dram_tensor(f"ag_out{i}", [2, 128, 4, 512], f16) for i in range(4)]

    with ExitStack() as ctx:
        tc = ctx.enter_context(tile.TileContext(nc))
        sres = ctx.enter_context(tc.tile_pool(name="res", bufs=1))
        swts = ctx.enter_context(tc.tile_pool(name="wts", bufs=2))
        stab = ctx.enter_context(tc.tile_pool(name="tab", bufs=2))
        sx = ctx.enter_context(tc.tile_pool(name="x", bufs=2))
        stmp = ctx.enter_context(tc.tile_pool(name="tmp", bufs=3))
        sp = ctx.enter_context(tc.tile_pool(name="p", bufs=3))
        sof = ctx.enter_context(tc.tile_pool(name="of", bufs=2))
        sout = ctx.enter_context(tc.tile_pool(name="out", bufs=2))
        ssm = ctx.enter_context(tc.tile_pool(name="sm", bufs=2))
        pps = ctx.enter_context(tc.tile_pool(name="ps", bufs=3, space="PSUM"))
        pac = ctx.enter_context(tc.tile_pool(name="ac", bufs=2, space="PSUM"))

        # ---- resident tiles
        qT_t = sres.tile([128, 4, T], f16, tag="qT")       # rope'd q, feature-major
        kT_t = sres.tile([128, 4, T], f16, tag="kT")
        ot_t = sres.tile([128, 4, T], f16, tag="ot")       # attention out, feature-major
        # fp8 v in DoubleRow pair layout: [keys, pair, sub(kt&1), head, 80pad]
        v8_t = sres.tile([128, NP, 2, HL, 80], f8, tag="v8")
        # fp16 v for the first two key tiles (early-row accuracy)
        v16_t = sres.tile([128, 2, HL, 65], f16, tag="v16")
        nc.vector.memset(v8_t[:, :, :, :, 64:65], 1.0)
        nc.vector.memset(v16_t[:, :, :, 64:65], 1.0)

        perm_t = sres.tile([128, 128], f16, tag="perm")
        tri_t = sres.tile([128, 128], f16, tag="tri")
        bq_t = sres.tile([128, 4], f32, tag="bq")
        bk_t = sres.tile([128, 4], f32, tag="bk")
        bvb_t = sres.tile([128, F], f16, tag="bvb")
        bpb_t = sres.tile([128, E // 2], f16, tag="bpb")
        ctab_t = stab.tile([128, T], f16, tag="tab")
        stabp_t = stab.tile([128, T], f16, tag="tab")

        wq_t = swts.tile([128, 4, KB, 128], f16, tag="wqk")
        wk_t = swts.tile([128, 4, KB, 128], f16, tag="wqk")
        wv_t = swts.tile([128, KB, F], f16, tag="w")
        wp_t = swts.tile([128, KB, E // 2], f16, tag="w")

        # startup: x chunk-0 + q/k weights split in halves on the sync
        # queue (first matmul chain can start after the first pieces);
        # tables + everything else on the gpsimd queue.
        x0_t = sx.tile([128, KB, 512], f16, tag="x")
        nc.sync.dma_start(out=x0_t[:, 0:4, :], in_=x16[:, 0, 0:4, :])
        nc.sync.dma_start(out=x0_t[:, 4:8, :], in_=x16[:, 0, 4:8, :])
        for f in range(4):
            nc.sync.dma_start(out=wq_t[:, f], in_=wq[:, f])
        for f in range(4):
            nc.scalar.dma_start(out=wk_t[:, f], in_=wk[:, f])

        nc.gpsimd.dma_start(out=ctab_t, in_=ctab_d[:, :])
        nc.gpsimd.dma_start(out=stabp_t, in_=stabp_d[:, :])
        nc.gpsimd.dma_start(out=perm_t, in_=perm_d[:, :])
        nc.gpsimd.dma_start(out=wv_t, in_=wv[:, :, :])
        nc.gpsimd.dma_start(out=tri_t, in_=tri_d[:, :])
        nc.gpsimd.dma_start(out=bq_t, in_=bq[:, :])
        nc.gpsimd.dma_start(out=bk_t, in_=bk[:, :])
        nc.gpsimd.dma_start(out=bvb_t, in_=bvb[:, :])
        nc.gpsimd.dma_start(out=bpb_t, in_=bpb[:, :])
        nc.gpsimd.dma_start(out=wp_t, in_=wproj[:, :, :])

        def load_x(tcx):
            x_t = sx.tile([128, KB, 512], f16, tag="x")
            nc.sync.dma_start(out=x_t, in_=x16[:, tcx, :, :])
            return x_t

        def qkv_chunk(tcx, x_t):
            """Generator: 8 q/k f-steps + 4 v-steps, yields between steps."""
            cs = slice(tcx * 512, (tcx + 1) * 512)
            pend = None  # (ps_p, dst, f, t1)

            def flush_perm():
                nonlocal pend
                if pend is None:
                    return
                ps_p, dst, f, t1 = pend
                nc.vector.tensor_add(dst[:, f, cs], t1[:, :], ps_p)
                pend = None

            for w_t, b_t, dst in ((wq_t, bq_t, qT_t), (wk_t, bk_t, kT_t)):
                for f in range(4):
                    ps2 = pps.tile([128, 1024], f32, tag="mm")
                    ps_q = ps2[:, 0:512]
                    ps_p = ps2[:, 512:1024]
                    for kb in range(KB):
                        nc.tensor.matmul(
                            ps_q,
                            w_t[:, f, kb, :],
                            x_t[:, kb, :],
                            start=(kb == 0), stop=(kb == KB - 1),
                        )
                    # bias add on gpsimd, rope muls on DVE
                    q16 = stmp.tile([128, 512], f16, tag="q16")
                    nc.vector.tensor_scalar_add(q16[:, :], ps_q, b_t[:, f:f + 1])
                    qs = stmp.tile([128, 512], f16, tag="qs")
                    nc.vector.tensor_mul(qs[:, :], q16[:, :], stabp_t[:, cs])
                    t1 = stmp.tile([128, 512], f16, tag="t1")
                    nc.vector.tensor_mul(t1[:, :], q16[:, :], ctab_t[:, cs])
                    nc.tensor.matmul(ps_p, perm_t[:, :], qs[:, :],
                                     start=True, stop=True)
                    flush_perm()
                    pend = (ps_p, dst, f, t1)
                    yield
            flush_perm()

            for tl in range(4):
                tt = tcx * 4 + tl
                ps2 = pps.tile([128, 1024], f32, tag="mm")
                ps_v = ps2[:, 0:512]
                for kb in range(KB):
                    nc.tensor.matmul(
                        ps_v,
                        x_t[:, kb, tl * 128:(tl + 1) * 128],
                        wv_t[:, kb, :],
                        start=(kb == 0), stop=(kb == KB - 1),
                    )
                ps_v3 = ps_v.rearrange("p (h d) -> p h d", h=HL)
                bv3 = bvb_t.rearrange("p (h d) -> p h d", h=HL)
                nc.vector.tensor_add(
                    v8_t[:, tt // 2, tt % 2, :, 0:D], ps_v3, bv3)
                if tt < 2:
                    nc.vector.tensor_add(
                        v16_t[:, tt, :, 0:D], ps_v3, bv3)
                yield

        def proj_chunk(gi):
            """Generator: 4 tl units, each split in two 4-kb half-chains."""
            q0, w = GROUPS[gi]
            of_t = sof.tile([128, 2, 4, 512], f16, tag="of")
            nc.sync.dma_start(
                out=of_t,
                in_=ag_out[gi].rearrange("s p a t -> p s a t"),
            )
            of_r = of_t.rearrange("p s a t -> p (s a) t")
            for tl in range(4):
                tt = q0 // 128 + tl
                o_st = sout.tile([128, E // 2], f32, tag="o")
                ps2 = pps.tile([128, 1024], f32, tag="mm")
                ps_pj = ps2[:, 0:512]
                for kb in range(KB):
                    nc.tensor.matmul(
                        ps_pj,
                        of_r[:, kb, tl * 128:(tl + 1) * 128],
                        wp_t[:, kb, :],
                        start=(kb == 0), stop=(kb == KB - 1),
                    )
                nc.vector.tensor_add(o_st[:, :], ps_pj, bpb_t[:, :])
                nc.scalar.dma_start(out=out_ext[tt * 128:(tt + 1) * 128, :],
                                    in_=o_st)
                yield

        def attn_group(gi):
            """Generator: yields after each (hp, kt-pair) step."""
            q0, w = GROUPS[gi]
            qs_sl = slice(q0, q0 + w)
            kt0 = q0 // 128
            nkt = kt0 + 4
            npair = nkt // 2

            def w0_of(kt):
                return max(kt - kt0, 0) * 128

            def emit_S(hp, kt):
                w0 = w0_of(kt)
                ps_s = pps.tile([128, 1024], f32, tag="mm")
                for half, bp in ((0, 0), (1, 64)):
                    nc.tensor.matmul(
                        ps_s[:, half * w + w0:half * w + w],
                        kT_t[bp:bp + 64, hp, kt * 128:(kt + 1) * 128],
                        qT_t[bp:bp + 64, hp, q0 + w0:q0 + w],
                        start=True, stop=True,
                    )
                return ps_s

            def emit_exp(hp, j, sub, ps_s, p8_t, p16_t):
                kt = 2 * j + sub
                w0 = w0_of(kt)
                src = ps_s.rearrange("p (h q) -> p h q", h=2)
                use16 = (gi == 0 and j == 0)
                dst_t = p16_t if use16 else p8_t
                nc.scalar.activation(
                    dst_t[:, :, sub, w0:512], src[:, :, w0:512],
                    mybir.ActivationFunctionType.Exp, scale=float(D) ** -0.5,
                )
                if kt >= kt0:  # diagonal tile: mask upper triangle
                    for h in range(2):
                        ms = slice(w0, w0 + 128)
                        nc.vector.tensor_mul(dst_t[:, h, sub, ms],
                                             dst_t[:, h, sub, ms], tri_t[:, :])
                if sub == 1 and kt > kt0:  # zero the odd member's gap
                    g0_, g1_ = w0_of(2 * j), w0
                    if g1_ > g0_:
                        nc.vector.memset(dst_t[:, :, 1, g0_:g1_], 0.0)

            def emit_pv(hp, j, ps_os, p8_t, p16_t):
                psA, psB = ps_os
                w0 = w0_of(2 * j)
                stop = (j == npair - 1)
                if gi == 0 and j == 0:
                    for kt in (0, 1):
                        kw0 = w0_of(kt)
                        for h, ps_o in ((0, psA), (1, psB)):
                            nc.tensor.matmul(
                                ps_o[:, kw0:512],
                                v16_t[:, kt, 2 * hp + h, :],
                                p16_t[:, h, kt, kw0:512],
                                start=(kt == 0), stop=(stop and kt == 1),
                            )
                else:
                    for h, ps_o in ((0, psA), (1, psB)):
                        nc.tensor.matmul(
                            ps_o[:, w0:512],
                            v8_t[:, j, :, 2 * hp + h, 0:65],
                            p8_t[:, h, :, w0:512],
                            start=(j == 0), stop=stop,
                            perf_mode=mybir.MatmulPerfMode.DoubleRow,
                        )

            def finish_hp(hp, ps_os):
                for bp, ps_o in ((0, ps_os[0]), (64, ps_os[1])):
                    sums_sb = ssm.tile([1, 512], f32, tag="sums")
                    nc.vector.tensor_copy(sums_sb[:, :], ps_o[D:D + 1, :])
                    recip = ssm.tile([1, 512], f32, tag="rc")
                    nc.vector.reciprocal_approx_fast(out=recip[:, :],
                                                     in_=sums_sb[:, :])
                    bc = ssm.tile([64, 512], f32, tag="bc")
                    nc.gpsimd.partition_broadcast(bc[:, :], recip[:, :])
                    nc.vector.tensor_mul(ot_t[bp:bp + 64, hp, qs_sl],
                                         ps_o[0:D, :], bc[:, :])
                # stage this head-pair's gathered strip
                nc.sync.dma_start(
                    out=ag_in[gi][:, hp, :],
                    in_=ot_t[:, hp, q0:q0 + w])
                if hp == 3:
                    nc.gpsimd.collective_compute(
                        "AllGather",
                        mybir.AluOpType.bypass,
                        ins=[ag_in[gi][:, :, :]],
                        outs=[ag_out[gi][:, :, :, :]],
                        replica_groups=[[0, 1], [2, 3], [4, 5], [6, 7]],
                    )

            for hp in range(HL // 2):
                ps_os = (pac.tile([65, 512], f32, tag="acc", name="oA"),
                         pac.tile([65, 512], f32, tag="acc", name="oB"))
                for j in range(npair):
                    p8_t = sp.tile([128, 2, 2, 512], f8, tag="p8", name="p8_t")
                    p16_t = None
                    if gi == 0 and j == 0:
                        p16_t = sp.tile([128, 2, 2, 512], f16, tag="p16",
                                        name="p16_t")
                    ps_e = emit_S(hp, 2 * j)
                    ps_o2 = emit_S(hp, 2 * j + 1)
                    emit_exp(hp, j, 0, ps_e, p8_t, p16_t)
                    emit_exp(hp, j, 1, ps_o2, p8_t, p16_t)
                    # fillers land between exp and PV so the PE has work
                    # while ACT runs the exp pipeline
                    yield
                    emit_pv(hp, j, ps_os, p8_t, p16_t)
                finish_hp(hp, ps_os)

        # ---------- interleaved schedule ----------
        # attention group g runs with qkv chunk g+1 and projection of
        # group g-1 woven between its pair-steps.
        for _ in qkv_chunk(0, x0_t):
            pass
        x_next = load_x(1)
        for gi in range(TC):
            filler_gens = []
            nfill = 0
            if gi + 1 < TC:
                filler_gens.append(qkv_chunk(gi + 1, x_next))
                nfill += 12
            if gi >= 1:
                filler_gens.append(proj_chunk(gi - 1))
                nfill += 4
            if gi + 2 < TC:
                x_next = load_x(gi + 2)
            npairs = 4 * 2 * (gi + 1)
            emitted = 0
            step = 0
            for _ in attn_group(gi):
                step += 1
                want = min(nfill, (step * nfill) // max(npairs - 2, 1))
                while emitted < want and filler_gens:
                    try:
                        next(filler_gens[0])
                        emitted += 1
                    except StopIteration:
                        filler_gens.pop(0)
            while filler_gens:
                try:
                    next(filler_gens[0])
                    emitted += 1
                except StopIteration:
                    filler_gens.pop(0)
        for _ in proj_chunk(3):
            pass

    nc.compile()
    return nc


_NC = None


def _get_nc():
    global _NC
    if _NC is None:
        _NC = build_nc()
    return _NC


def _host_prep(x, Wqkv, bqkv, Wproj, bproj):
    """Build the 8 per-core input maps."""
    x = np.asarray(x, np.float32)
    Wqkv = np.asarray(Wqkv, np.float32)
    bqkv = np.asarray(bqkv, np.float32)
    Wproj = np.asarray(Wproj, np.float32)
    bproj = np.asarray(bproj, np.float32)

    perm_idx = np.concatenate([np.arange(0, D, 2), np.arange(1, D, 2)])

    inv_freq = 1.0 / ROPE_THETA ** (np.arange(0, D, 2, dtype=np.float32) / D)
    freqs = np.arange(T, dtype=np.float32)[:, None] * inv_freq[None, :]
    cosf = np.cos(freqs).T.astype(np.float32)  # (32, T)
    sinf = np.sin(freqs).T.astype(np.float32)
    ctab = np.tile(cosf, (4, 1)).astype(np.float16)                 # (128, T)
    stab = np.concatenate([-sinf, sinf, -sinf, sinf], 0).astype(np.float16)
    pi = np.array([(m // 64) * 64 + (m % 64 + 32) % 64 for m in range(128)])
    stabp = stab[pi]

    pmat = np.zeros((128, 128), np.float16)
    for m in range(128):
        pmat[pi[m], m] = 1.0

    tri = (np.arange(128)[:, None] <= np.arange(128)[None, :]).astype(np.float16)

    maps = []
    for c in range(8):
        b, g = c // 2, c % 2
        heads = np.arange(8 * g, 8 * g + 8)
        qcols = (heads[:, None] * D + perm_idx[None, :]).ravel()
        vcols = (heads[:, None] * D + np.arange(D)[None, :]).ravel()

        xb = x[b].astype(np.float16)                      # (T, E)
        x16 = xb.reshape(TC, 512, KB, 128).transpose(3, 0, 2, 1)

        def wmat(cols):
            wm = Wqkv[:, cols].astype(np.float16)          # (E, F)
            return wm.reshape(KB, 128, F).transpose(1, 0, 2)

        def wmat_f(cols):
            # [128, 4f, KB, 128]
            wm = Wqkv[:, cols].astype(np.float16)          # (E, F)
            return wm.reshape(KB, 128, 4, 128).transpose(1, 2, 0, 3)

        # wproj rows ordered to match proj contraction blocks kb=(s, hp):
        # feature(s, hp, p) = (8s + 2*hp + (p>=64))*64 + p%64
        s_i, a_i, p_i = np.meshgrid(np.arange(2), np.arange(4), np.arange(128),
                                    indexing="ij")
        feat = (8 * s_i + 2 * a_i + (p_i >= 64)) * 64 + p_i % 64
        wp = Wproj[:, g * 512:(g + 1) * 512].astype(np.float16)
        wp16 = wp[feat.reshape(8, 128)].transpose(1, 0, 2)

        maps.append({
            "x16": np.ascontiguousarray(x16),
            "wq": np.ascontiguousarray(wmat_f(qcols)),
            "wk": np.ascontiguousarray(wmat_f(E + qcols)),
            "wv": np.ascontiguousarray(wmat(2 * E + vcols)),
            "wproj": np.ascontiguousarray(wp16),
            "bq": np.ascontiguousarray(bqkv[qcols].reshape(4, 128).T.astype(np.float32)),
            "bk": np.ascontiguousarray(bqkv[E + qcols].reshape(4, 128).T.astype(np.float32)),
            "bvb": np.ascontiguousarray(
                np.tile(bqkv[2 * E + vcols].astype(np.float16)[None, :], (128, 1))),
            "bpb": np.ascontiguousarray(
                np.tile(bproj[g * 512:(g + 1) * 512].astype(np.float16)[None, :], (128, 1))),
            "ctab": ctab,
            "stabp": np.ascontiguousarray(stabp),
            "perm": pmat,
            "tri": tri,
        })
    return maps


def kernel(x, Wqkv, bqkv, Wproj, bproj):
    nc = _get_nc()
    in_maps = _host_prep(x, Wqkv, bqkv, Wproj, bproj)
    res = run_bass_kernel_spmd(nc, in_maps, list(range(8)))
    out = np.empty((B, T, E), np.float32)
    for b in range(B):
        out[b, :, :E // 2] = res.results[2 * b]["out"]
        out[b, :, E // 2:] = res.results[2 * b + 1]["out"]
    return out


if __name__ == "__main__":
    rng = np.random.default_rng(0)
    x = rng.standard_normal((B, T, E), dtype=np.float32)
    Wqkv = rng.standard_normal((E, 3 * E), dtype=np.float32) * 0.02
    bqkv = rng.standard_normal((3 * E,), dtype=np.float32) * 0.02
    Wproj = rng.standard_normal((E, E), dtype=np.float32) * 0.02
    bproj = rng.standard_normal((E,), dtype=np.float32) * 0.02
    o = kernel(x=x, Wqkv=Wqkv, bqkv=bqkv, Wproj=Wproj, bproj=bproj)
    print("out", o.shape, o.dtype, float(np.abs(o).max()))
